# revision 17
# baseline (speedup 1.0000x reference)
"""Trainium2 Bass kernel for nn_CustomModel_52338471469275 (dense MLP).

Computes out = relu(input @ (S*THETA)^T + bias) @ weight + bias2
  input  [2048, 8192] f32
  S,THETA[1024, 8192] f32   (fused on host into W1 = S*THETA)
  weight [1024, 1024] f32
  out    [2048, 1024] f32

Sharding over 8 NeuronCores: 4 batch groups (512 rows each) x 2 hidden
halves (512 of the 1024 hidden units each).  Core (i, j) computes

  fT_ij  = relu(W1[jblk] @ x[iblk]^T + bias[jblk])          # [512, 512]
  outT_p = weight[jblk]^T @ fT_ij                           # [1024, 512]

i.e. a partial (contraction-split) second GEMM.  The host sums the two
j-partials per batch group, transposes, and adds bias2.  No on-device
collectives needed.

All matmul operands are cast to bf16 on the host (fp32 PSUM accumulation
on device).  Measured end-to-end relative error vs the fp32 reference is
~3e-3 (absmax-relative), from bf16 operand rounding.
"""

import os
import sys

import numpy as np

if "/opt/trn_rl_repo" not in sys.path:
    sys.path.insert(0, "/opt/trn_rl_repo")

import ml_dtypes

import concourse.bass as bass
import concourse.tile as tile
from concourse import mybir
from concourse._compat import checkenv
from concourse.bass_utils import run_bass_kernel_spmd

B, O, I = 2048, 1024, 8192
R, C = 4, 2                 # batch groups x hidden halves
BS, OS = B // R, O // C     # 512, 512
P = 128
N = BS                      # moving free dim per matmul
KT1 = I // P                # 64 k-tiles, GEMM1
MT1 = OS // P               # 4 m-tiles, GEMM1
KT2 = OS // P               # 4 k-tiles, GEMM2
MT2 = O // P                # 8 m-tiles, GEMM2

BF16 = mybir.dt.bfloat16
F32 = mybir.dt.float32

# k-tiles per slab DMA for GEMM1 (first blocks small so the PE starts early)
SCHED = [1, 1, 2] + [4] * 15
assert sum(SCHED) == KT1


def _blockize(aT):
    """Rewrite [8192, W] so each SCHED block of QK k-tiles is stored p-major
    ([P, QK, W] C-order): one contiguous QK*W-element descriptor per SBUF
    partition instead of QK separate rows."""
    out = np.empty_like(aT)
    kt0 = 0
    for QK in SCHED:
        blk = aT[kt0 * P : (kt0 + QK) * P]
        out[kt0 * P : (kt0 + QK) * P] = (
            blk.reshape(QK, P, -1).transpose(1, 0, 2).reshape(QK * P, -1)
        )
        kt0 += QK
    return out

_CACHE = {}
LAST_RESULTS = None  # BassKernelResults of the most recent run (for test.py)


def _split_multi_waits(nc, max_waits=1):
    """This container's walrus codegen rejects instructions carrying more
    than one semaphore wait ("Too many sync wait commands", CoreV3GenImpl).
    Tile's kernel-tail drain aggregates several; hoist the extras onto
    preceding same-engine NoOps (identical semantics: engines execute their
    stream in order)."""
    for fn in nc.m.functions:
        for blk in fn.blocks:
            new_insts = []
            for inst in blk.instructions:
                si = inst.sync_info
                waits = list(si.on_wait) if si and si.on_wait else []
                if len(waits) > max_waits:
                    extra, keep = waits[:-max_waits], waits[-max_waits:]
                    for k, w in enumerate(extra):
                        new_insts.append(
                            mybir.InstNoOp(
                                name=f"{inst.name}_wsplit{k}",
                                engine=inst.engine,
                                ins=[],
                                outs=[],
                                sync_info=mybir.SyncInfo(on_wait=[w], on_update=[]),
                            )
                        )
                    inst.sync_info = mybir.SyncInfo(
                        on_wait=keep,
                        on_update=list(si.on_update) if si.on_update else [],
                    )
                new_insts.append(inst)
            blk.instructions = new_insts


def _build_nc() -> bass.Bass:
    nc = bass.Bass()
    xT = nc.declare_dram_parameter("xT", [I, BS], BF16, isOutput=False)
    w1T = nc.declare_dram_parameter("w1T", [I, OS], BF16, isOutput=False)
    b1 = nc.declare_dram_parameter("b1", [P, MT1], F32, isOutput=False)
    w2 = nc.declare_dram_parameter("w2", [OS, O], BF16, isOutput=False)
    outT = nc.declare_dram_parameter("outT", [O, BS], F32, isOutput=True)

    with tile.TileContext(nc) as tc:
        with (
            tc.tile_pool(name="const", bufs=1) as const,
            tc.tile_pool(name="xs", bufs=5) as xpool,
            tc.tile_pool(name="ws", bufs=5) as wpool,
            tc.tile_pool(name="fp", bufs=1) as fpool,
            tc.tile_pool(name="op", bufs=3) as opool,
            tc.tile_pool(name="ps1", bufs=1, space="PSUM") as ps1,
            tc.tile_pool(name="ps2", bufs=2, space="PSUM") as ps2,
        ):
            # PE warm-up: ~3.5us of dummy matmuls while the first slabs are
            # still in flight, so the HAM clock gate opens (1.2 -> 2.4 GHz)
            # before the real accumulation begins.
            warm = const.tile([P, N], BF16)
            nc.vector.memset(warm[:], 0.0)
            wps = ps2.tile([P, 2, N], F32, tag="p2g")
            for _ in range(8):
                nc.tensor.matmul(wps[:, 0, :], warm[:, :P], warm[:],
                                 start=True, stop=True)

            # GEMM1: logitsT[m1blk, :] += W1T[ktblk, m1blk]^T @ xT[ktblk, :]
            # x slabs issued from SP (sync), w1 slabs from ACT (scalar) so
            # neither engine's descriptor generation is the bottleneck.
            ps = ps1.tile([P, MT1, N], F32)  # 4 PSUM banks, one per m1
            kt0 = 0
            for kb, QK in enumerate(SCHED):
                r0 = kt0 * P
                # host stores each slab block p-major ([P, QK, N] C-order),
                # so every SBUF partition line is one QK*N*2-byte contiguous
                # DMA descriptor instead of QK separate 1 KB rows
                xs = xpool.tile([P, 4, N], BF16, tag="xs")
                nc.sync.dma_start(
                    xs[:, :QK, :],
                    xT[r0 : r0 + QK * P, :].rearrange("(p q) n -> p q n", p=P),
                )
                ws = wpool.tile([P, 4, OS], BF16, tag="ws")
                nc.scalar.dma_start(
                    ws[:, :QK, :],
                    w1T[r0 : r0 + QK * P, :].rearrange("(p q) n -> p q n", p=P),
                )
                for q in range(QK):
                    for m1 in range(MT1):
                        nc.tensor.matmul(
                            ps[:, m1, :],
                            ws[:, q, m1 * P : (m1 + 1) * P],
                            xs[:, q, :],
                            start=(kt0 + q == 0),
                            stop=(kt0 + q == KT1 - 1),
                        )
                kt0 += QK

            # constants for the second GEMM (SP has slack between slab
            # triggers; avoiding gpsimd skips its costly SWDGE drain)
            b1_t = const.tile([P, MT1], F32)
            nc.sync.dma_start(b1_t[:], b1[:])
            w2_sb = const.tile([P, KT2, O], BF16)
            for kt in range(KT2):
                nc.sync.dma_start(w2_sb[:, kt, :], w2[kt * P : (kt + 1) * P, :])

            # bias + relu, cast to bf16
            f_sb = fpool.tile([P, KT2, N], BF16)
            for m1 in range(MT1):
                nc.scalar.activation(
                    f_sb[:, m1, :],
                    ps[:, m1, :],
                    mybir.ActivationFunctionType.Relu,
                    bias=b1_t[:, m1 : m1 + 1],
                )

            # GEMM2 (partial over this core's hidden half):
            # outT[m2blk, :] = sum_kt2 w2[kt2blk, m2blk]^T @ fT[kt2blk, :]
            # kt-outer over m2-pairs: the first matmuls only need f_sb[:,0,:]
            # (first relu), so GEMM2 overlaps the relu pipeline instead of
            # waiting for all four.
            for g in range(MT2 // 2):
                p2g = ps2.tile([P, 2, N], F32, tag="p2g")
                for kt in range(KT2):
                    for m2 in range(2):
                        mm = g * 2 + m2
                        nc.tensor.matmul(
                            p2g[:, m2, :],
                            w2_sb[:, kt, mm * P : (mm + 1) * P],
                            f_sb[:, kt, :],
                            start=(kt == 0),
                            stop=(kt == KT2 - 1),
                        )
                for m2 in range(2):
                    mm = g * 2 + m2
                    ot = opool.tile([P, N], F32)
                    # alternate copy engines so bank release isn't gated on
                    # one serialized copy queue
                    if m2 == 0:
                        nc.vector.tensor_copy(ot[:], p2g[:, m2, :])
                    else:
                        nc.scalar.activation(
                            ot[:], p2g[:, m2, :],
                            mybir.ActivationFunctionType.Copy,
                        )
                    nc.sync.dma_start(outT[mm * P : (mm + 1) * P, :], ot[:])

    _split_multi_waits(nc)
    return nc


def kernel(input, S, THETA, bias, weight, bias2):
    global LAST_RESULTS
    if "nc" not in _CACHE:
        _CACHE["nc"] = _build_nc()
    nc = _CACHE["nc"]

    bf16 = ml_dtypes.bfloat16
    input = np.asarray(input, dtype=np.float32)
    W1 = np.asarray(S, dtype=np.float32) * np.asarray(THETA, dtype=np.float32)
    bias = np.asarray(bias, dtype=np.float32)
    weight = np.asarray(weight, dtype=np.float32)
    bias2 = np.asarray(bias2, dtype=np.float32)

    xT_g = [
        _blockize(np.ascontiguousarray(input[i * BS : (i + 1) * BS, :].T).astype(bf16))
        for i in range(R)
    ]
    w1T_g = [
        _blockize(np.ascontiguousarray(W1[j * OS : (j + 1) * OS, :].T).astype(bf16))
        for j in range(C)
    ]
    b1_g = [
        np.ascontiguousarray(bias[j * OS : (j + 1) * OS].reshape(MT1, P).T)
        for j in range(C)
    ]
    w2_g = [weight[j * OS : (j + 1) * OS, :].astype(bf16) for j in range(C)]

    in_maps = []
    for i in range(R):
        for j in range(C):
            in_maps.append(
                {"xT": xT_g[i], "w1T": w1T_g[j], "b1": b1_g[j], "w2": w2_g[j]}
            )

    res = run_bass_kernel_spmd(
        nc,
        in_maps,
        core_ids=list(range(R * C)),
        trace=checkenv("BASS_TRACE"),
    )
    LAST_RESULTS = res

    out = np.empty((B, O), dtype=np.float32)
    for i in range(R):
        acc = res.results[i * C]["outT"].astype(np.float32)
        for j in range(1, C):
            acc = acc + res.results[i * C + j]["outT"]
        out[i * BS : (i + 1) * BS, :] = acc.T
    out += bias2[None, :]
    return out


# revision 19
# speedup vs baseline: 1.0353x; 1.0353x over previous
"""Trainium2 Bass kernel for nn_CustomModel_52338471469275 (dense MLP).

Computes out = relu(input @ (S*THETA)^T + bias) @ weight + bias2
  input  [2048, 8192] f32
  S,THETA[1024, 8192] f32   (fused on host into W1 = S*THETA)
  weight [1024, 1024] f32
  out    [2048, 1024] f32

Sharding over 8 NeuronCores: 4 batch groups (512 rows each) x 2 hidden
halves (512 of the 1024 hidden units each).  Core (i, j) computes

  fT_ij  = relu(W1[jblk] @ x[iblk]^T + bias[jblk])          # [512, 512]
  outT_p = weight[jblk]^T @ fT_ij                           # [1024, 512]

i.e. a partial (contraction-split) second GEMM.  The host sums the two
j-partials per batch group, transposes, and adds bias2.  No on-device
collectives needed.

All matmul operands are cast to bf16 on the host (fp32 PSUM accumulation
on device).  Measured end-to-end relative error vs the fp32 reference is
~3e-3 (absmax-relative), from bf16 operand rounding.
"""

import os
import sys

import numpy as np

if "/opt/trn_rl_repo" not in sys.path:
    sys.path.insert(0, "/opt/trn_rl_repo")

import ml_dtypes

import concourse.bass as bass
import concourse.tile as tile
from concourse import mybir
from concourse._compat import checkenv
from concourse.bass_utils import run_bass_kernel_spmd

B, O, I = 2048, 1024, 8192
R, C = 4, 2                 # batch groups x hidden halves
BS, OS = B // R, O // C     # 512, 512
P = 128
N = BS                      # moving free dim per matmul
KT1 = I // P                # 64 k-tiles, GEMM1
MT1 = OS // P               # 4 m-tiles, GEMM1
KT2 = OS // P               # 4 k-tiles, GEMM2
MT2 = O // P                # 8 m-tiles, GEMM2

BF16 = mybir.dt.bfloat16
F32 = mybir.dt.float32

# k-tiles per slab DMA for GEMM1 (first blocks small so the PE starts early)
SCHED = [1, 1, 2] + [4] * 15
assert sum(SCHED) == KT1


def _blockize(aT):
    """Rewrite [8192, W] so each SCHED block of QK k-tiles is stored p-major
    ([P, QK, W] C-order): one contiguous QK*W-element descriptor per SBUF
    partition instead of QK separate rows."""
    out = np.empty_like(aT)
    kt0 = 0
    for QK in SCHED:
        blk = aT[kt0 * P : (kt0 + QK) * P]
        out[kt0 * P : (kt0 + QK) * P] = (
            blk.reshape(QK, P, -1).transpose(1, 0, 2).reshape(QK * P, -1)
        )
        kt0 += QK
    return out

_CACHE = {}
LAST_RESULTS = None  # BassKernelResults of the most recent run (for test.py)


def _split_multi_waits(nc, max_waits=1):
    """This container's walrus codegen rejects instructions carrying more
    than one semaphore wait ("Too many sync wait commands", CoreV3GenImpl).
    Tile's kernel-tail drain aggregates several; hoist the extras onto
    preceding same-engine NoOps (identical semantics: engines execute their
    stream in order)."""
    for fn in nc.m.functions:
        for blk in fn.blocks:
            new_insts = []
            for inst in blk.instructions:
                si = inst.sync_info
                waits = list(si.on_wait) if si and si.on_wait else []
                if len(waits) > max_waits:
                    extra, keep = waits[:-max_waits], waits[-max_waits:]
                    for k, w in enumerate(extra):
                        new_insts.append(
                            mybir.InstNoOp(
                                name=f"{inst.name}_wsplit{k}",
                                engine=inst.engine,
                                ins=[],
                                outs=[],
                                sync_info=mybir.SyncInfo(on_wait=[w], on_update=[]),
                            )
                        )
                    inst.sync_info = mybir.SyncInfo(
                        on_wait=keep,
                        on_update=list(si.on_update) if si.on_update else [],
                    )
                new_insts.append(inst)
            blk.instructions = new_insts


def _build_nc() -> bass.Bass:
    nc = bass.Bass()
    xT = nc.declare_dram_parameter("xT", [I, BS], BF16, isOutput=False)
    w1T = nc.declare_dram_parameter("w1T", [I, OS], BF16, isOutput=False)
    b1 = nc.declare_dram_parameter("b1", [P, MT1], F32, isOutput=False)
    w2 = nc.declare_dram_parameter("w2", [OS, O], BF16, isOutput=False)
    # bf16 partials: halves the output write traffic in the serial tail;
    # the host reduces the two j-partials in fp32
    outT = nc.declare_dram_parameter("outT", [O, BS], BF16, isOutput=True)

    with tile.TileContext(nc) as tc:
        with (
            tc.tile_pool(name="const", bufs=1) as const,
            tc.tile_pool(name="xs", bufs=5) as xpool,
            tc.tile_pool(name="ws", bufs=5) as wpool,
            tc.tile_pool(name="fp", bufs=1) as fpool,
            tc.tile_pool(name="op", bufs=3) as opool,
            tc.tile_pool(name="ps1", bufs=1, space="PSUM") as ps1,
            tc.tile_pool(name="ps2", bufs=2, space="PSUM") as ps2,
        ):
            # PE warm-up: ~3.5us of dummy matmuls while the first slabs are
            # still in flight, so the HAM clock gate opens (1.2 -> 2.4 GHz)
            # before the real accumulation begins.
            warm = const.tile([P, N], BF16)
            nc.vector.memset(warm[:], 0.0)
            wps = ps2.tile([P, 2, N], F32, tag="p2g")
            for _ in range(8):
                nc.tensor.matmul(wps[:, 0, :], warm[:, :P], warm[:],
                                 start=True, stop=True)

            # GEMM1: logitsT[m1blk, :] += W1T[ktblk, m1blk]^T @ xT[ktblk, :]
            # x slabs issued from SP (sync), w1 slabs from ACT (scalar) so
            # neither engine's descriptor generation is the bottleneck.
            ps = ps1.tile([P, MT1, N], F32)  # 4 PSUM banks, one per m1
            kt0 = 0
            for kb, QK in enumerate(SCHED):
                r0 = kt0 * P
                # host stores each slab block p-major ([P, QK, N] C-order),
                # so every SBUF partition line is one QK*N*2-byte contiguous
                # DMA descriptor instead of QK separate 1 KB rows
                xs = xpool.tile([P, 4, N], BF16, tag="xs")
                nc.sync.dma_start(
                    xs[:, :QK, :],
                    xT[r0 : r0 + QK * P, :].rearrange("(p q) n -> p q n", p=P),
                )
                ws = wpool.tile([P, 4, OS], BF16, tag="ws")
                nc.scalar.dma_start(
                    ws[:, :QK, :],
                    w1T[r0 : r0 + QK * P, :].rearrange("(p q) n -> p q n", p=P),
                )
                for q in range(QK):
                    for m1 in range(MT1):
                        nc.tensor.matmul(
                            ps[:, m1, :],
                            ws[:, q, m1 * P : (m1 + 1) * P],
                            xs[:, q, :],
                            start=(kt0 + q == 0),
                            stop=(kt0 + q == KT1 - 1),
                        )
                kt0 += QK

            # constants for the second GEMM (SP has slack between slab
            # triggers; avoiding gpsimd skips its costly SWDGE drain)
            b1_t = const.tile([P, MT1], F32)
            nc.sync.dma_start(b1_t[:], b1[:])
            w2_sb = const.tile([P, KT2, O], BF16)
            for kt in range(KT2):
                nc.sync.dma_start(w2_sb[:, kt, :], w2[kt * P : (kt + 1) * P, :])

            # bias + relu, cast to bf16
            f_sb = fpool.tile([P, KT2, N], BF16)
            for m1 in range(MT1):
                nc.scalar.activation(
                    f_sb[:, m1, :],
                    ps[:, m1, :],
                    mybir.ActivationFunctionType.Relu,
                    bias=b1_t[:, m1 : m1 + 1],
                )

            # GEMM2 (partial over this core's hidden half):
            # outT[m2blk, :] = sum_kt2 w2[kt2blk, m2blk]^T @ fT[kt2blk, :]
            # kt-outer over m2-pairs: the first matmuls only need f_sb[:,0,:]
            # (first relu), so GEMM2 overlaps the relu pipeline instead of
            # waiting for all four.
            for g in range(MT2 // 2):
                p2g = ps2.tile([P, 2, N], F32, tag="p2g")
                for kt in range(KT2):
                    for m2 in range(2):
                        mm = g * 2 + m2
                        nc.tensor.matmul(
                            p2g[:, m2, :],
                            w2_sb[:, kt, mm * P : (mm + 1) * P],
                            f_sb[:, kt, :],
                            start=(kt == 0),
                            stop=(kt == KT2 - 1),
                        )
                for m2 in range(2):
                    mm = g * 2 + m2
                    ot = opool.tile([P, N], BF16)
                    nc.vector.tensor_copy(ot[:], p2g[:, m2, :])
                    nc.sync.dma_start(outT[mm * P : (mm + 1) * P, :], ot[:])

    _split_multi_waits(nc)
    return nc


def kernel(input, S, THETA, bias, weight, bias2):
    global LAST_RESULTS
    if "nc" not in _CACHE:
        _CACHE["nc"] = _build_nc()
    nc = _CACHE["nc"]

    bf16 = ml_dtypes.bfloat16
    input = np.asarray(input, dtype=np.float32)
    W1 = np.asarray(S, dtype=np.float32) * np.asarray(THETA, dtype=np.float32)
    bias = np.asarray(bias, dtype=np.float32)
    weight = np.asarray(weight, dtype=np.float32)
    bias2 = np.asarray(bias2, dtype=np.float32)

    xT_g = [
        _blockize(np.ascontiguousarray(input[i * BS : (i + 1) * BS, :].T).astype(bf16))
        for i in range(R)
    ]
    w1T_g = [
        _blockize(np.ascontiguousarray(W1[j * OS : (j + 1) * OS, :].T).astype(bf16))
        for j in range(C)
    ]
    b1_g = [
        np.ascontiguousarray(bias[j * OS : (j + 1) * OS].reshape(MT1, P).T)
        for j in range(C)
    ]
    w2_g = [weight[j * OS : (j + 1) * OS, :].astype(bf16) for j in range(C)]

    in_maps = []
    for i in range(R):
        for j in range(C):
            in_maps.append(
                {"xT": xT_g[i], "w1T": w1T_g[j], "b1": b1_g[j], "w2": w2_g[j]}
            )

    res = run_bass_kernel_spmd(
        nc,
        in_maps,
        core_ids=list(range(R * C)),
        trace=checkenv("BASS_TRACE"),
    )
    LAST_RESULTS = res

    out = np.empty((B, O), dtype=np.float32)
    for i in range(R):
        acc = res.results[i * C]["outT"].astype(np.float32)
        for j in range(1, C):
            acc = acc + res.results[i * C + j]["outT"]
        out[i * BS : (i + 1) * BS, :] = acc.T
    out += bias2[None, :]
    return out
